# revision 1
# baseline (speedup 1.0000x reference)
"""Trainium2 Bass kernel for nn_ClassLayer_56564719289025.

Reference computation:  y = mean(|W|) * (x @ sign(W).T)
  x: [8192, 4096] f32, W: [4096, 4096] f32 -> y: [8192, 4096] f32

Strategy (8 NeuronCores):
  - Data-parallel over x rows: each core computes a 1024-row shard of y.
  - W is replicated; sign(W) is exactly representable in bf16, so the
    matmul runs in bf16 (x rounded to bf16; error ~1e-3 relative).
  - scale = mean(|W|) is computed on-device: each core abs-sums a
    distinct 512-column slice of W^T (1/8 of W), then a 512-byte
    AllReduce across the 8 cores combines the partials.
  - Host-side prep is layout-only: bf16 cast + transposes + sharding
    (the contraction dim must be the SBUF partition dim on both matmul
    operands, so both x and W are shipped transposed).

Per-core loop: x^T shard resident in SBUF (bf16, 8.4MB); W^T streamed
in 8 o-blocks of 512 columns; ACT engine converts W tiles to sign tiles;
PE runs 2048 accumulating [128x128]@[128x512] bf16 matmuls; DVE evicts
PSUM with a fused multiply by the broadcast scale.
"""

import numpy as np
import ml_dtypes

import concourse.bacc as bacc
import concourse.bass_isa as bass_isa
import concourse.mybir as mybir
import concourse.tile as tile
from concourse.bass_utils import run_bass_kernel_spmd

TOKENS, D_IN, D_OUT, N_CORES = 8192, 4096, 4096, 8
P = 128            # SBUF partitions / matmul contraction tile
OB = 512           # output-column block (one PSUM bank at fp32)
R_SHARD = TOKENS // N_CORES   # 1024 rows per core
KO = D_IN // P                # 32 contraction tiles
NB = D_OUT // OB              # 8 o-blocks
RT = R_SHARD // P             # 8 row tiles per core
SCAN_W = D_OUT // N_CORES     # 512-column scan slice per core
INV_N = 1.0 / (D_IN * D_OUT)  # exactly 2**-24

bf16 = mybir.dt.bfloat16
fp32 = mybir.dt.float32


def _emit(tc, xT, wT, wS, y, part, red, reps=1):
    nc = tc.nc
    xT3 = xT.rearrange("(ko p) r -> p ko r", p=P)   # [128, 32, 1024]
    wT3 = wT.rearrange("(ko p) o -> p ko o", p=P)   # [128, 32, 4096]
    wS3 = wS.rearrange("(ko p) o -> p ko o", p=P)   # [128, 32, 512]
    y3 = y.rearrange("(rt p) o -> p rt o", p=P)     # [128, 8, 4096]

    with (
        tc.tile_pool(name="xpool", bufs=1) as xpool,
        tc.tile_pool(name="spool", bufs=2) as spool,
        tc.tile_pool(name="wmain", bufs=8) as wmain,
        tc.tile_pool(name="wscan", bufs=2) as wscan,
        tc.tile_pool(name="scpool", bufs=1) as scpool,
        tc.tile_pool(name="ypool", bufs=12) as ypool,
        tc.tile_pool(name="psum", bufs=8, space="PSUM") as psum,
    ):
        for _ in range(reps):
            def produce_S(b):
                S_b = spool.tile([P, KO, OB], bf16, tag="S")
                for k in range(KO):
                    w_t = wmain.tile([P, OB], bf16, tag="w")
                    nc.sync.dma_start(w_t[:], wT3[:, k, b * OB:(b + 1) * OB])
                    nc.scalar.sign(S_b[:, k, :], w_t[:])
                return S_b

            # --- x and block-0 W loaded k-sliced, interleaved so the k-outer
            # block-0 matmuls can start after just one (x[k], S0[k]) pair and
            # consume W at 1 tile per 8 matmuls (well under DMA rate) ---
            x_sb = xpool.tile([P, KO, R_SHARD], bf16, tag="x")
            S0 = spool.tile([P, KO, OB], bf16, tag="S")
            for k in range(KO):
                nc.sync.dma_start(x_sb[:, k, :], xT3[:, k, :])
                w_t = wmain.tile([P, OB], bf16, tag="w")
                nc.sync.dma_start(w_t[:], wT3[:, k, 0:OB])
                nc.scalar.sign(S0[:, k, :], w_t[:])

            acc = scpool.tile([P, KO], fp32, tag="acc")

            def scan_j(j):
                ws_t = wscan.tile([P, 8, OB], bf16, tag="ws")
                nc.sync.dma_start(ws_t[:], wS3[:, j * 8:(j + 1) * 8, :])
                nc.vector.tensor_reduce(
                    acc[:, j * 8:(j + 1) * 8], ws_t[:],
                    axis=mybir.AxisListType.X, op=mybir.AluOpType.add,
                    apply_absolute_value=True,
                )

            for j in range(4):
                scan_j(j)

            # --- finish scale: partition reduce, cross-core AllReduce ---
            acc1 = scpool.tile([P, 1], fp32, tag="acc1")
            nc.vector.tensor_reduce(
                acc1[:], acc[:], axis=mybir.AxisListType.X, op=mybir.AluOpType.add
            )
            accs = scpool.tile([P, 1], fp32, tag="accs")
            nc.vector.tensor_scalar_mul(accs[:], acc1[:], INV_N)
            par_t = scpool.tile([P, 1], fp32, tag="par")
            nc.gpsimd.partition_all_reduce(
                par_t[:], accs[:], channels=P, reduce_op=bass_isa.ReduceOp.add
            )
            nc.sync.dma_start(part[:], par_t[:])
            nc.gpsimd.collective_compute(
                "AllReduce", mybir.AluOpType.add,
                [list(range(N_CORES))], [part[:]], [red[:]],
            )
            scale_sb = scpool.tile([P, 1], fp32, tag="scale")
            nc.sync.dma_start(scale_sb[:], red[:])

            # --- block 1 prefetch ---
            S1 = produce_S(1)

            def evict(ps, r, b):
                # two-step eviction: DVE copy frees the PSUM bank without
                # waiting on scale; the scale multiply binds later
                y_t = ypool.tile([P, OB], fp32, tag="y")
                nc.vector.tensor_copy(out=y_t[:], in_=ps[:])
                nc.vector.tensor_scalar_mul(y_t[:], y_t[:], scale_sb[:])
                nc.sync.dma_start(y3[:, r, b * OB:(b + 1) * OB], y_t[:])

            # --- block 0: k-outer over 8 concurrent PSUM banks, so the PE
            # starts on the first (x[k], S0[k]) pair and tracks DMA supply ---
            ps0 = [
                psum.tile([P, OB], fp32, tag="ps", name=f"ps0_{r}")
                for r in range(RT)
            ]
            for k in range(KO):
                for r in range(RT):
                    nc.tensor.matmul(
                        ps0[r][:],
                        lhsT=x_sb[:, k, r * P:(r + 1) * P],
                        rhs=S0[:, k, :],
                        start=(k == 0),
                        stop=(k == KO - 1),
                    )
            for r in range(RT):
                evict(ps0[r], r, 0)

            # --- blocks 1..7: r-inner, k-accumulate per group ---
            for b in range(1, NB):
                S_b = S1 if b == 1 else produce_S(b)
                for r in range(RT):
                    ps = psum.tile([P, OB], fp32, tag="ps")
                    for k in range(KO):
                        nc.tensor.matmul(
                            ps[:],
                            lhsT=x_sb[:, k, r * P:(r + 1) * P],
                            rhs=S_b[:, k, :],
                            start=(k == 0),
                            stop=(k == KO - 1),
                        )
                    evict(ps, r, b)


def build(reps=1):
    nc = bacc.Bacc(
        "TRN2", target_bir_lowering=False, debug=False, num_devices=N_CORES
    )
    xT = nc.dram_tensor("xT", [D_IN, R_SHARD], bf16, kind="ExternalInput").ap()
    wT = nc.dram_tensor("wT", [D_IN, D_OUT], bf16, kind="ExternalInput").ap()
    wS = nc.dram_tensor("wscan", [D_IN, SCAN_W], bf16, kind="ExternalInput").ap()
    y = nc.dram_tensor("y", [R_SHARD, D_OUT], fp32, kind="ExternalOutput").ap()
    part = nc.dram_tensor("part", [P, 1], fp32, kind="Internal").ap()
    red = nc.dram_tensor("red", [P, 1], fp32, kind="Internal", addr_space="Shared").ap()

    with tile.TileContext(nc) as tc:
        _emit(tc, xT, wT, wS, y, part, red, reps=reps)
    nc.compile()
    return nc


_NC_CACHE = {}


def _get_nc(reps=1):
    if reps not in _NC_CACHE:
        _NC_CACHE[reps] = build(reps)
    return _NC_CACHE[reps]


def _make_in_maps(x, weight):
    xb = np.asarray(x).astype(ml_dtypes.bfloat16)
    wb = np.asarray(weight).astype(ml_dtypes.bfloat16)
    xTb = np.ascontiguousarray(xb.T)   # [D_IN, TOKENS]
    wTb = np.ascontiguousarray(wb.T)   # [D_IN, D_OUT]
    in_maps = []
    for c in range(N_CORES):
        in_maps.append({
            "xT": np.ascontiguousarray(xTb[:, c * R_SHARD:(c + 1) * R_SHARD]),
            "wT": wTb,
            "wscan": np.ascontiguousarray(wTb[:, c * SCAN_W:(c + 1) * SCAN_W]),
        })
    return in_maps


def kernel(x, weight):
    x = np.asarray(x)
    weight = np.asarray(weight)
    assert x.shape == (TOKENS, D_IN), x.shape
    assert weight.shape == (D_OUT, D_IN), weight.shape
    in_maps = _make_in_maps(x, weight)
    nc = _get_nc(1)
    last_exc = None
    for attempt in range(3):
        try:
            res = run_bass_kernel_spmd(nc, in_maps, core_ids=list(range(N_CORES)))
            break
        except Exception as e:  # transient NRT device errors — retry
            last_exc = e
            import time as _time

            _time.sleep(2.0 * (attempt + 1))
    else:
        raise last_exc
    return np.concatenate(
        [res.results[c]["y"] for c in range(N_CORES)], axis=0
    ).astype(np.float32)



# revision 2
# speedup vs baseline: 1.1938x; 1.1938x over previous
"""Trainium2 Bass kernel for nn_ClassLayer_56564719289025 — all-fp8 DoubleRow.

Reference:  y = mean(|W|) * (x @ sign(W).T)
  x: [8192, 4096] f32, W: [4096, 4096] f32 -> y: [8192, 4096] f32

Strategy (8 NeuronCores, data-parallel over x rows):
  - All matmuls are fp8e4 with perf_mode=DoubleRow (2 fp8 weights/cell,
    256-deep contraction per instruction, ~2.4x bf16 throughput measured).
  - x ships as x8 = e4m3(x^T) (~2.7e-2 rel quantization error alone).
    A correction pass over the first 18 of 32 k-subtiles accumulates
    r8' @ (sign/32) into the same PSUM, where r8' = e4m3(32*(x - x8)) is
    computed on-device from a bf16 32*x shard; the 32x scaling keeps r8'
    out of fp8-denormal range (denormal operands slow the PE).  Net rel
    error 1.76e-2 < 2e-2 gate.
  - W ships as w8 = e4m3(2^13 * W^T), column-rotated by 512*core so each
    core's first o-block is also its |W|-scan slice: sign tiles and the
    mean(|W|) scan share one DMA stream.  scale = AllReduce of per-core
    partial sums; host un-rotates the gathered output (layout-only).
  - Per-block pipeline: W-pair DMA -> ACT sign (fp8) -> DVE /32 copy for
    the correction signs; PE runs 25 DR matmuls per 128-row chain across
    8 PSUM banks; DVE evicts with fused scale multiply (POOL engine
    applies scale for the first 2 blocks so eviction never waits on the
    AllReduce).
"""

import numpy as np
import ml_dtypes

import concourse.bacc as bacc
import concourse.bass_isa as bass_isa
import concourse.mybir as mybir
import concourse.tile as tile
from concourse.bass_utils import run_bass_kernel_spmd

TOKENS, D_IN, D_OUT, N_CORES = 8192, 4096, 4096, 8
P = 128
OB = 512
R_SHARD = TOKENS // N_CORES    # 1024 rows per core
KO = D_IN // P                 # 32 contraction subtiles
KP = KO // 2                   # 16 subtile pairs
NB = D_OUT // OB               # 8 o-blocks
RT = R_SHARD // P              # 8 row tiles per core
NCORR = 18                     # corrected k-subtiles
NCP = NCORR // 2               # 9 corrected pairs
W_EXP = 13                     # W pre-scale 2**13 (max |w8| ~222 < 240)
INV_N = 1.0 / (D_IN * D_OUT * (1 << W_EXP))

fp8 = mybir.dt.float8e4
bf16 = mybir.dt.bfloat16
fp32 = mybir.dt.float32
DR = mybir.MatmulPerfMode.DoubleRow


def _emit(tc, x8d, xbd, w8d, y, part, red, reps=1):
    nc = tc.nc
    x8d4 = x8d.rearrange("(kp t p) r -> p kp t r", p=P, t=2)   # [128,16,2,1024]
    xbd4 = xbd.rearrange("(kp t p) r -> p kp t r", p=P, t=2)   # [128,9,2,1024]
    w8d4 = w8d.rearrange("(kp t p) o -> p kp t o", p=P, t=2)   # [128,16,2,4096]
    y3 = y.rearrange("(rt p) o -> p rt o", p=P)                # [128,8,4096]

    with (
        tc.tile_pool(name="xpool", bufs=1) as xpool,
        tc.tile_pool(name="spool", bufs=2) as spool,
        tc.tile_pool(name="wpool", bufs=6) as wpool,
        tc.tile_pool(name="scpool", bufs=1) as scpool,
        tc.tile_pool(name="ypool", bufs=18) as ypool,
        tc.tile_pool(name="psum", bufs=8, space="PSUM") as psum,
    ):
        for _ in range(reps):
            x8_sb = xpool.tile([P, KP, 2, R_SHARD], fp8, tag="x8")
            xb_sb = xpool.tile([P, NCP, 2, R_SHARD], bf16, tag="xb")
            r8_sb = xpool.tile([P, NCP, 2, R_SHARD], fp8, tag="r8")
            ws_acc = scpool.tile([P, KP], fp32, tag="acc")

            def w_pipe(b, j, S, Sc, scan=False):
                """DMA one W pair tile, sign it, make correction signs."""
                w_t = wpool.tile([P, 2, OB], fp8, tag="w", name=f"w{b}_{j}")
                nc.sync.dma_start(w_t[:], w8d4[:, j, :, b * OB:(b + 1) * OB])
                nc.scalar.sign(S[:, j, :, :], w_t[:])
                if scan:
                    nc.vector.tensor_reduce(
                        ws_acc[:, j:j + 1], w_t[:],
                        axis=mybir.AxisListType.XY, op=mybir.AluOpType.add,
                        apply_absolute_value=True)
                if j < NCP:
                    nc.vector.tensor_scalar_mul(
                        Sc[:, j, :, :], S[:, j, :, :], 1.0 / 32.0)

            def s_tiles(b):
                S = spool.tile([P, KP, 2, OB], fp8, tag="s8", name=f"S{b}")
                Sc = spool.tile([P, NCP, 2, OB], fp8, tag="s8c", name=f"Sc{b}")
                return S, Sc

            # ---- startup: block-0 W pairs (+|W| scan) interleaved with x8 ----
            S0, Sc0 = s_tiles(0)
            for j in range(KP):
                w_pipe(0, j, S0, Sc0, scan=True)
                nc.sync.dma_start(x8_sb[:, j, :, :], x8d4[:, j, :, :])

            # ---- scale = mean|W| (per-core slice, AllReduce across cores) ----
            acc1 = scpool.tile([P, 1], fp32, tag="acc1")
            nc.vector.tensor_reduce(
                acc1[:], ws_acc[:], axis=mybir.AxisListType.X,
                op=mybir.AluOpType.add)
            accs = scpool.tile([P, 1], fp32, tag="accs")
            nc.vector.tensor_scalar_mul(accs[:], acc1[:], INV_N)
            par_t = scpool.tile([P, 1], fp32, tag="par")
            nc.gpsimd.partition_all_reduce(
                par_t[:], accs[:], channels=P, reduce_op=bass_isa.ReduceOp.add)
            nc.sync.dma_start(part[:], par_t[:])
            nc.gpsimd.collective_compute(
                "AllReduce", mybir.AluOpType.add,
                [list(range(N_CORES))], [part[:]], [red[:]])
            scale_sb = scpool.tile([P, 1], fp32, tag="scale")
            nc.sync.dma_start(scale_sb[:], red[:])

            # ---- xb32 + r8' = 32*(x-x8) on DVE, interleaved with the
            # block-1 W pipeline so neither starves the other ----
            S1, Sc1 = s_tiles(1)
            for j in range(KP):
                if j < NCP:
                    nc.sync.dma_start(xb_sb[:, j, :, :], xbd4[:, j, :, :])
                    nc.vector.scalar_tensor_tensor(
                        r8_sb[:, j, :, :], x8_sb[:, j, :, :], -32.0,
                        xb_sb[:, j, :, :],
                        op0=mybir.AluOpType.mult, op1=mybir.AluOpType.add)
                w_pipe(1, j, S1, Sc1)

            # ---- main loop over o-blocks ----
            for b in range(NB):
                if b == 0:
                    S, Sc = S0, Sc0
                elif b == 1:
                    S, Sc = S1, Sc1
                else:
                    S, Sc = s_tiles(b)
                    for j in range(KP):
                        w_pipe(b, j, S, Sc)
                pss = [psum.tile([P, OB], fp32, tag="ps", name=f"ps_{b}_{r}")
                       for r in range(RT)]
                for j in range(KP):
                    for r in range(RT):
                        nc.tensor.matmul(
                            pss[r][:],
                            lhsT=x8_sb[:, j, :, r * P:(r + 1) * P],
                            rhs=S[:, j, :, :],
                            start=(j == 0), stop=False, perf_mode=DR)
                for j in range(NCP):
                    for r in range(RT):
                        nc.tensor.matmul(
                            pss[r][:],
                            lhsT=r8_sb[:, j, :, r * P:(r + 1) * P],
                            rhs=Sc[:, j, :, :],
                            start=False, stop=(j == NCP - 1), perf_mode=DR)
                for r in range(RT):
                    y_t = ypool.tile([P, OB], fp32, tag="y", name=f"y_{b}_{r}")
                    if b < 2:
                        nc.vector.tensor_copy(out=y_t[:], in_=pss[r][:])
                        nc.gpsimd.tensor_scalar_mul(y_t[:], y_t[:], scale_sb[:])
                    else:
                        nc.vector.tensor_scalar_mul(
                            y_t[:], pss[r][:], scale_sb[:])
                    nc.sync.dma_start(y3[:, r, b * OB:(b + 1) * OB], y_t[:])


def build(reps=1):
    nc = bacc.Bacc(
        "TRN2", target_bir_lowering=False, debug=False, num_devices=N_CORES)
    x8d = nc.dram_tensor("x8", [D_IN, R_SHARD], fp8, kind="ExternalInput").ap()
    xbd = nc.dram_tensor("xb32", [NCORR * P, R_SHARD], bf16,
                         kind="ExternalInput").ap()
    w8d = nc.dram_tensor("w8", [D_IN, D_OUT], fp8, kind="ExternalInput").ap()
    y = nc.dram_tensor("y", [R_SHARD, D_OUT], fp32, kind="ExternalOutput").ap()
    part = nc.dram_tensor("part", [P, 1], fp32, kind="Internal").ap()
    red = nc.dram_tensor("red", [P, 1], fp32, kind="Internal",
                         addr_space="Shared").ap()
    with tile.TileContext(nc) as tc:
        _emit(tc, x8d, xbd, w8d, y, part, red, reps=reps)
    nc.compile()
    return nc


_NC_CACHE = {}


def _get_nc(reps=1):
    if reps not in _NC_CACHE:
        _NC_CACHE[reps] = build(reps)
    return _NC_CACHE[reps]


def _make_in_maps(x, weight):
    x = np.asarray(x, dtype=np.float32)
    weight = np.asarray(weight, dtype=np.float32)
    xT = np.ascontiguousarray(x.T)                       # [D_IN, TOKENS]
    x8T = xT.astype(ml_dtypes.float8_e4m3)
    xb32T = (xT[:NCORR * P] * 32.0).astype(ml_dtypes.bfloat16)
    w8T = np.ascontiguousarray(
        weight.T * float(1 << W_EXP)).astype(ml_dtypes.float8_e4m3)
    in_maps = []
    for c in range(N_CORES):
        rs = slice(c * R_SHARD, (c + 1) * R_SHARD)
        in_maps.append({
            "x8": np.ascontiguousarray(x8T[:, rs]),
            "xb32": np.ascontiguousarray(xb32T[:, rs]),
            # rotate o-columns so block 0 == this core's |W|-scan slice
            "w8": np.ascontiguousarray(np.roll(w8T, -c * OB, axis=1)),
        })
    return in_maps


def kernel(x, weight):
    x = np.asarray(x)
    weight = np.asarray(weight)
    assert x.shape == (TOKENS, D_IN), x.shape
    assert weight.shape == (D_OUT, D_IN), weight.shape
    in_maps = _make_in_maps(x, weight)
    nc = _get_nc(1)
    last_exc = None
    for attempt in range(3):
        try:
            res = run_bass_kernel_spmd(nc, in_maps, core_ids=list(range(N_CORES)))
            break
        except Exception as e:
            last_exc = e
            import time as _time

            _time.sleep(2.0 * (attempt + 1))
    else:
        raise last_exc
    # un-rotate each core's output columns, then stack rows
    return np.concatenate(
        [np.roll(res.results[c]["y"], c * OB, axis=1) for c in range(N_CORES)],
        axis=0,
    ).astype(np.float32)
